# revision 19
# baseline (speedup 1.0000x reference)
"""Multi-head causal self-attention (B=2, T=2048, C=1024, H=16) on 8 trn2 cores.

Sharding: data-parallel over batch (2) x tensor-parallel over heads (4 groups
of 4 heads). Core c handles batch b=c//4, head group g=c%4.

Key structure (per core):
  - x is pre-transposed and pre-cast to f16 on the host (window-major
    layout) so there is no on-device transpose phase and every DMA is
    cast-free. fp8 was evaluated for the QKV projection (DoubleRow) but its
    quantization noise exceeds the 2e-2 budget, so everything stays f16.
  - Attention in S^T orientation (k on partitions, q free), f16 operands.
    The causal mask is a PE add-matmul into the S accumulation group
    (-200 strict-upper-tri stationary x identity moving).
  - Rowsums come from a ones-column appended to V; recip on DVE;
    partition_broadcast on Pool.
  - Output projection (row-parallel Wproj, bias via a 65th ones-row in oT)
    is pipelined per 512-row sub-chunk with attention of the next sub-chunk,
    feeding chunked ReduceScatter collectives that overlap compute.
  - All DMAs are cast-free (host pre-casts) and issue via HWDGE (nc.sync).
"""

import os

import numpy as np

import concourse.bacc as bacc
import concourse.bass as bass
import concourse.mybir as mybir
import concourse.tile as tile
from concourse.bass_utils import run_bass_kernel_spmd

DEBUG = bool(int(os.environ.get("KERNEL_DEBUG", "0")))

F32 = mybir.dt.float32
F16 = mybir.dt.float16

B, T, C, H = 2, 2048, 1024, 16
HPC = 4                 # heads per core
HD = 64                 # head dim
CG = HPC * 3 * HD       # 768 qkv cols per core
KC = 8                  # f16 contraction chunks (128 channels each)
TT = T // 128           # 16 k tiles
NSC = T // 512          # 4 q sub-chunks
N_CORES = 8
EXP_SCALE = 0.125

# reduce-scatter groups as (row_start, row_end); each core keeps len/4 rows
RS_GROUPS = [(0, 1024), (1024, 1536), (1536, 2048)]


def _build():
    nc = bacc.Bacc(None, target_bir_lowering=False)

    x16_in = nc.dram_tensor("x16", [128, 4 * KC * 512], F16, kind="ExternalInput")
    w16_in = nc.dram_tensor("w16", [128, KC * CG], F16, kind="ExternalInput")
    qkb_in = nc.dram_tensor("qkb", [128, 4], F32, kind="ExternalInput")
    vb_in = nc.dram_tensor("vb", [1, 256], F16, kind="ExternalInput")
    wpa_in = nc.dram_tensor("wpa", [65, HPC * C], F16, kind="ExternalInput")
    out_part = nc.dram_tensor("out_part", [T // 4, C], F16, kind="ExternalOutput")

    partial_d = nc.dram_tensor("partial_d", [T, C], F16)
    rsout_d = [
        nc.dram_tensor(f"rsout_d{i}", [(r1 - r0) // 4, C], F16)
        for i, (r0, r1) in enumerate(RS_GROUPS)
    ]

    dbg = {}
    if DEBUG:
        dbg["qkT"] = nc.dram_tensor("dbg_qkT", [128, 4 * T], F16, kind="ExternalOutput")
        dbg["v_aug"] = nc.dram_tensor(
            "dbg_v_aug", [128, TT * HPC * 65], F16, kind="ExternalOutput"
        )
        dbg["oT"] = nc.dram_tensor("dbg_oT", [65, HPC * 512 * 2], F16, kind="ExternalOutput")
        dbg["partial"] = nc.dram_tensor("dbg_partial", [T, C], F16, kind="ExternalOutput")

    with tile.TileContext(nc) as tc:
        with (
            tc.tile_pool(name="cpool", bufs=1) as cpool,
            tc.tile_pool(name="main", bufs=1) as main,
            tc.tile_pool(name="stage", bufs=1) as stage,
            tc.tile_pool(name="ps", bufs=1, space="PSUM") as ps,
        ):
            # ---------------- constants ----------------
            ones_row = cpool.tile([1, 512], F16)
            nc.vector.memset(ones_row[:], 1.0)
            vb_bc = cpool.tile([128, 256], F16)
            # mask stationary: mstat[f, p] = -200 where p > f else 0
            mstat = cpool.tile([128, 128], F16)
            nc.gpsimd.memset(mstat[:], -200.0)
            nc.gpsimd.affine_select(
                out=mstat[:], in_=mstat[:],
                compare_op=mybir.AluOpType.is_ge, fill=0.0,
                base=-1, pattern=[[1, 128]], channel_multiplier=-1,
            )
            # mask moving: identity
            mmov = cpool.tile([128, 128], F16)
            nc.gpsimd.memset(mmov[:], 0.0)
            nc.gpsimd.affine_select(
                out=mmov[:], in_=mmov[:],
                compare_op=mybir.AluOpType.not_equal, fill=1.0,
                base=0, pattern=[[-1, 128]], channel_multiplier=1,
            )

            # ---------------- persistent tensors ----------------
            x16 = main.tile([128, 4 * KC * 512], F16)   # [w][kc][512]
            w16 = main.tile([128, KC * CG], F16)        # [kc][768]
            qkb = main.tile([128, 4], F32)
            vb = main.tile([1, 256], F16)
            wpa = main.tile([65, HPC * C], F16)
            qkT = main.tile([128, 4 * T], F16)             # [Q01;Q23;K01;K23] x T
            v_aug = main.tile([128, TT * HPC * 65], F16)   # per (tt,h): 64 V + ones col
            oT_sb = [
                main.tile([65, HPC * 512], F16, name=f"oT_sb{i}") for i in range(2)
            ]

            nc.vector.memset(v_aug[:], 1.0)  # ones columns give softmax rowsums
            for buf in oT_sb:
                nc.vector.memset(buf[64:65, :], 1.0)

            # ---------------- input DMAs (all cast-free, HWDGE) ----------
            w16_r = w16[:].rearrange("p (kc m) -> p kc m", kc=KC)
            w16_in_r = w16_in[:].rearrange("p (kc m) -> p kc m", kc=KC)
            nc.sync.dma_start(w16_r[:, 0:4, 512:768], w16_in_r[:, 0:4, 512:768])
            nc.sync.dma_start(x16[:, 0:2048], x16_in[:, 0:2048])
            nc.sync.dma_start(w16_r[:, 4:8, 512:768], w16_in_r[:, 4:8, 512:768])
            nc.sync.dma_start(x16[:, 2048:4096], x16_in[:, 2048:4096])
            nc.sync.dma_start(vb[:], vb_in[:])
            nc.gpsimd.partition_broadcast(vb_bc[:], vb[:])
            nc.sync.dma_start(qkb[:], qkb_in[:])
            nc.sync.dma_start(w16_r[:, :, 0:512], w16_in_r[:, :, 0:512])
            for w in range(1, 4):
                nc.sync.dma_start(
                    x16[:, w * 4096 : (w + 1) * 4096],
                    x16_in[:, w * 4096 : (w + 1) * 4096],
                )
            nc.sync.dma_start(wpa[:], wpa_in[:])

            # ---------------- emit helpers ----------------
            def x16_w(w):
                # [128, kc, 512] view of window w
                return x16[:, w * 4096 : (w + 1) * 4096].rearrange(
                    "p (kc t) -> p kc t", kc=KC
                )


            def emit_v(tt):
                w, tloc = divmod(tt, 4)
                pp = ps.tile([128, 1024], F32, tag="mm", bufs=3)
                for kc in range(KC):
                    nc.tensor.matmul(
                        pp[:, 0:256],
                        x16_w(w)[:, kc, tloc * 128 : (tloc + 1) * 128],
                        w16_r[:, kc, 512:768],
                        start=(kc == 0), stop=(kc == KC - 1),
                    )
                vt = v_aug[:, tt * HPC * 65 : (tt + 1) * HPC * 65].rearrange(
                    "p (h c) -> p h c", c=65
                )[:, :, 0:64]
                nc.vector.scalar_tensor_tensor(
                    out=vt,
                    in0=pp[:, 0:256].rearrange("p (h c) -> p h c", c=64),
                    scalar=1.0,
                    in1=vb_bc[:].rearrange("p (h c) -> p h c", c=64),
                    op0=mybir.AluOpType.mult,
                    op1=mybir.AluOpType.add,
                )

            def emit_qk(i, tch, evac_on_act=False):
                pp0 = ps.tile([128, 1024], F32, tag="mm", bufs=3)
                pp = pp0[:, 0:512]
                for kc in range(KC):
                    nc.tensor.matmul(
                        pp,
                        w16_r[:, kc, i * 128 : (i + 1) * 128],
                        x16_w(tch)[:, kc, :],
                        start=(kc == 0), stop=(kc == KC - 1),
                    )
                dst = qkT[:, i * T + tch * 512 : i * T + (tch + 1) * 512]
                if evac_on_act:
                    nc.scalar.activation(
                        dst, pp, mybir.ActivationFunctionType.Identity,
                        bias=qkb[:, i : i + 1],
                    )
                else:
                    nc.vector.tensor_scalar_add(dst, pp, qkb[:, i : i + 1])

            # per (sc, h) attention state
            ot_tiles = {}
            rs_tiles = {}

            def emit_att_head(sc, h, fillers=None):
                qT = qkT[64 * (h % 2) : 64 * (h % 2) + 64, (h // 2) * T : (h // 2 + 1) * T]
                kT = qkT[64 * (h % 2) : 64 * (h % 2) + 64, (2 + h // 2) * T : (3 + h // 2) * T]
                oT_ps = ps.tile([65, 512], F32, tag="ot", bufs=2)
                ot_tiles[(sc, h)] = oT_ps
                n_kj = (sc + 1) * 4
                npairs = n_kj // 2

                def pair_layout(p):
                    # [(bank_off, q_off, cols, kj), ...]; pack both k tiles
                    # into one PSUM bank when their columns fit (saves exp
                    # span and a bank)
                    kj0, kj1 = 2 * p, 2 * p + 1
                    qo0 = max(0, kj0 * 128 - sc * 512)
                    qo1 = max(0, kj1 * 128 - sc * 512)
                    c0, c1 = 512 - qo0, 512 - qo1
                    if c0 + c1 <= 512:
                        return [(0, qo0, c0, kj0), (c0, qo1, c1, kj1)], c0 + c1
                    return [(0, qo0, c0, kj0), (512, qo1, c1, kj1)], 512 + c1

                def emit_s_pair(p):
                    layout, span = pair_layout(p)
                    one_bank = layout[1][0] < 512
                    st = ps.tile([128, 1024], F32, tag="mm", bufs=3)
                    pt = stage.tile([128, 1024], F16, tag="pt", bufs=4)
                    for idx, (boff, q_off, cols, kj) in enumerate(layout):
                        diag = kj >= sc * 4
                        first = idx == 0 or not one_bank
                        last_in_group = (not one_bank) or idx == 1
                        nc.tensor.matmul(
                            st[:, boff : boff + cols],
                            kT[:, kj * 128 : (kj + 1) * 128],
                            qT[:, sc * 512 + q_off : (sc + 1) * 512],
                            start=first,
                            stop=(not diag) and last_in_group,
                            skip_group_check=not first,
                        )
                        if diag:
                            nc.tensor.matmul(
                                st[:, boff : boff + 128], mstat[:], mmov[:],
                                start=False, stop=last_in_group,
                                skip_group_check=True,
                            )
                    # one exp covering both halves (cols between valid ranges
                    # hold stale PSUM; the pt garbage there is never read)
                    nc.scalar.activation(
                        pt[:, :span], st[:, :span],
                        mybir.ActivationFunctionType.Exp,
                        scale=EXP_SCALE,
                    )
                    return pt, layout

                def emit_pv_pair(p, pt, layout):
                    for boff, q_off, cols, kj in layout:
                        vv = v_aug[:, (kj * HPC + h) * 65 : (kj * HPC + h + 1) * 65]
                        nc.tensor.matmul(
                            oT_ps[:, q_off:512],
                            vv,
                            pt[:, boff : boff + cols],
                            start=(kj == 0),
                            stop=(kj == n_kj - 1),
                        )

                # software pipeline: emit S(p+1) before PV(p) so PE always has
                # matmul work queued while exp(p) completes on ACT; fillers
                # (independent matmul units) slot in to absorb ACT-bound gaps
                fillers = list(fillers or [])
                pend = emit_s_pair(0)
                for p in range(1, npairs):
                    nxt = emit_s_pair(p)
                    if fillers:
                        fillers.pop(0)()
                    emit_pv_pair(p - 1, *pend)
                    pend = nxt
                emit_pv_pair(npairs - 1, *pend)
                for f in fillers:
                    f()
                # normalize chain for this head (DVE + Pool), frees oT_ps
                rs = stage.tile([1, 512], F32, tag="rs", bufs=4)
                nc.vector.tensor_copy(rs[:], oT_ps[64:65, :])
                recip = stage.tile([1, 512], F32, tag="recip", bufs=4)
                nc.vector.reciprocal_approx_fast(recip[:], rs[:])
                bc = stage.tile([64, 512], F32, tag="bc", bufs=4)
                nc.gpsimd.partition_broadcast(bc[:], recip[:])
                nc.vector.tensor_mul(
                    oT_sb[sc % 2][0:64, h * 512 : (h + 1) * 512],
                    oT_ps[0:64, :], bc[:],
                )

            def emit_proj_piece(sc, j, split_evac=False):
                # project rows [sc*512 + j*128, +128)
                r0 = sc * 512 + j * 128
                oT_cur = oT_sb[sc % 2]
                pp = ps.tile([128, 1024], F32, tag="mm", bufs=3)
                for nch in range(2):
                    for hh in range(HPC):
                        nc.tensor.matmul(
                            pp[:, nch * 512 : (nch + 1) * 512],
                            oT_cur[:, hh * 512 + j * 128 : hh * 512 + (j + 1) * 128],
                            wpa[:, hh * C + nch * 512 : hh * C + (nch + 1) * 512],
                            start=(hh == 0), stop=(hh == HPC - 1),
                        )
                pst = stage.tile([128, 1024], F16, tag="pst", bufs=3)
                if split_evac:
                    nc.vector.tensor_copy(pst[:, 0:512], pp[:, 0:512])
                    nc.scalar.activation(
                        pst[:, 512:1024], pp[:, 512:1024],
                        mybir.ActivationFunctionType.Copy,
                    )
                    nc.sync.dma_start(partial_d[r0 : r0 + 128, 0:512], pst[:, 0:512])
                    nc.sync.dma_start(partial_d[r0 : r0 + 128, 512:1024], pst[:, 512:1024])
                else:
                    nc.vector.tensor_copy(pst[:], pp[:])
                    nc.sync.dma_start(partial_d[r0 : r0 + 128, :], pst[:])

            def emit_rs(gi):
                r0, r1 = RS_GROUPS[gi]
                nc.gpsimd.collective_compute(
                    "ReduceScatter",
                    mybir.AluOpType.add,
                    replica_groups=[[0, 1, 2, 3], [4, 5, 6, 7]],
                    ins=[partial_d[r0:r1, :]],
                    outs=[rsout_d[gi][:]],
                )

            def emit_out_copy(gi):
                # deferred to the tail: an out-copy waiting on its collective
                # must not sit in the in-order HWDGE queue ahead of partial
                # writes that later collectives depend on
                r0, r1 = RS_GROUPS[gi]
                og = sum((b1 - b0) // 4 for (b0, b1) in RS_GROUPS[:gi])
                ln4 = (r1 - r0) // 4
                nc.sync.dma_start(out_part[og : og + ln4, :], rsout_d[gi][:])

            # ---------------- schedule ----------------
            for tt in range(8):
                emit_v(tt)
            for tch in (0, 1):
                for i in range(4):
                    emit_qk(i, tch, evac_on_act=True)

            # att sc0 (V8/V9 fill the tiny heads' chain latency)
            att0_fill = [None, None, [lambda: emit_v(8)], [lambda: emit_v(9)]]
            for h in range(HPC):
                emit_att_head(0, h, att0_fill[h])
            # att sc1 with proj(sc0) interleaved between heads
            att1_fill = [[lambda: emit_v(10)], [lambda: emit_v(11)], None, None]
            for h in range(HPC):
                emit_att_head(1, h, att1_fill[h])
                emit_proj_piece(0, h)

            # fin1 (stall filled by two tch2 QK units), then RS group 0
            emit_qk(0, 2)
            emit_qk(1, 2)
            for j in range(4):
                emit_proj_piece(1, j)
            emit_rs(0)
            emit_qk(2, 2)
            emit_qk(3, 2)

            # att2 with att3's prerequisites sprinkled into exp-wait slots
            att2_fill = [
                [lambda: emit_v(12), lambda: emit_qk(0, 3)],
                [lambda: emit_v(13), lambda: emit_qk(1, 3)],
                [lambda: emit_v(14)],
                [lambda: emit_v(15)],
            ]
            for h in range(HPC):
                emit_att_head(2, h, att2_fill[h])
            # fin2 start (two tch3 QK units fill the chain), rest in att3
            emit_qk(2, 3)
            emit_qk(3, 3)
            emit_proj_piece(2, 0)
            emit_proj_piece(2, 1)

            for h in range(HPC):
                emit_att_head(3, h)
                if h == 0:
                    emit_proj_piece(2, 2)
                elif h == 1:
                    emit_proj_piece(2, 3)
                    emit_rs(1)
            for j in range(4):
                emit_proj_piece(3, j, split_evac=(j == 3))
            emit_rs(2)
            for gi in range(len(RS_GROUPS)):
                emit_out_copy(gi)

            if DEBUG:
                nc.sync.dma_start(dbg["qkT"][:], qkT[:])
                nc.sync.dma_start(dbg["v_aug"][:], v_aug[:])
                nc.sync.dma_start(dbg["oT"][:, 0 : HPC * 512], oT_sb[0][:])
                nc.sync.dma_start(dbg["oT"][:, HPC * 512 :], oT_sb[1][:])
                nc.sync.dma_start(dbg["partial"][:], partial_d[:])

    nc.finalize()
    return nc


_NC = None


def _get_nc():
    global _NC
    if _NC is None:
        _NC = _build()
    return _NC


def _perm_qkv(w):
    # (..., h*192 + t*64 + c) -> (..., t*256 + h*64 + c)
    s = w.shape[:-1]
    return np.ascontiguousarray(
        w.reshape(*s, HPC, 3, HD).swapaxes(-3, -2).reshape(*s, CG)
    )


def _make_in_maps(x, Wqkv, bqkv, Wproj, bproj):
    x = np.asarray(x, dtype=np.float32)
    Wqkv = np.asarray(Wqkv, dtype=np.float32)
    bqkv = np.asarray(bqkv, dtype=np.float32)
    Wproj = np.asarray(Wproj, dtype=np.float32)
    bproj = np.asarray(bproj, dtype=np.float32)

    in_maps = []
    for c in range(N_CORES):
        b, g = divmod(c, 4)
        # x16: [p, w, kc, t] window-major transposed layout
        xT = x[b].T  # (C, T)
        x16 = np.ascontiguousarray(
            xT.reshape(KC, 128, 4, 512).transpose(1, 2, 0, 3).reshape(128, -1)
        ).astype(np.float16)
        wp_ = _perm_qkv(Wqkv[:, g * CG : (g + 1) * CG])
        w16 = np.ascontiguousarray(
            wp_.reshape(KC, 128, CG).transpose(1, 0, 2).reshape(128, -1)
        ).astype(np.float16)
        bq = _perm_qkv(bqkv[g * CG : (g + 1) * CG])
        qkb = np.ascontiguousarray(bq[:512].reshape(4, 128).T).astype(np.float32)
        vb = bq[512:768].reshape(1, 256).astype(np.float16)
        wpa = np.zeros((65, HPC * C), np.float32)
        for hh in range(HPC):
            wpa[0:64, hh * C : (hh + 1) * C] = Wproj[
                g * 256 + hh * 64 : g * 256 + (hh + 1) * 64, :
            ]
        if g == 0:
            wpa[64, 0:C] = bproj
        in_maps.append(
            {
                "x16": x16,
                "w16": w16,
                "qkb": qkb,
                "vb": vb,
                "wpa": wpa.astype(np.float16),
            }
        )
    return in_maps


def _run(in_maps, trace=False):
    nc = _get_nc()
    return run_bass_kernel_spmd(nc, in_maps, list(range(N_CORES)), trace=trace)


def kernel(x, Wqkv, bqkv, Wproj, bproj):
    in_maps = _make_in_maps(x, Wqkv, bqkv, Wproj, bproj)
    res = _run(in_maps)
    out = np.empty((B, T, C), np.float32)
    for c in range(N_CORES):
        b, g = divmod(c, 4)
        op = res.results[c]["out_part"].astype(np.float32)
        og = 0
        for r0, r1 in RS_GROUPS:
            ln4 = (r1 - r0) // 4
            out[b, r0 + g * ln4 : r0 + (g + 1) * ln4, :] = op[og : og + ln4]
            og += ln4
    return out


# revision 22
# speedup vs baseline: 1.0122x; 1.0122x over previous
"""Multi-head causal self-attention (B=2, T=2048, C=1024, H=16) on 8 trn2 cores.

Sharding: data-parallel over batch (2) x tensor-parallel over heads (4 groups
of 4 heads). Core c handles batch b=c//4, head group g=c%4.

Key structure (per core):
  - x is pre-transposed and pre-cast to f16 on the host (window-major
    layout) so there is no on-device transpose phase and every DMA is
    cast-free. fp8 was evaluated for the QKV projection (DoubleRow) but its
    quantization noise exceeds the 2e-2 budget, so everything stays f16.
  - Attention in S^T orientation (k on partitions, q free), f16 operands.
    The causal mask is a PE add-matmul into the S accumulation group
    (-200 strict-upper-tri stationary x identity moving).
  - Rowsums come from a ones-column appended to V; recip on DVE;
    partition_broadcast on Pool.
  - Output projection (row-parallel Wproj, bias via a 65th ones-row in oT)
    is pipelined per 512-row sub-chunk with attention of the next sub-chunk,
    feeding chunked ReduceScatter collectives that overlap compute.
  - All DMAs are cast-free (host pre-casts) and issue via HWDGE (nc.sync).
"""

import os

import numpy as np

import concourse.bacc as bacc
import concourse.bass as bass
import concourse.mybir as mybir
import concourse.tile as tile
from concourse.bass_utils import run_bass_kernel_spmd

DEBUG = bool(int(os.environ.get("KERNEL_DEBUG", "0")))

F32 = mybir.dt.float32
F16 = mybir.dt.float16

B, T, C, H = 2, 2048, 1024, 16
HPC = 4                 # heads per core
HD = 64                 # head dim
CG = HPC * 3 * HD       # 768 qkv cols per core
KC = 8                  # f16 contraction chunks (128 channels each)
TT = T // 128           # 16 k tiles
NSC = T // 512          # 4 q sub-chunks
N_CORES = 8
EXP_SCALE = 0.125

# reduce-scatter groups as (row_start, row_end); each core keeps len/4 rows
RS_GROUPS = [(0, 1024), (1024, 1536), (1536, 2048)]


def _build():
    nc = bacc.Bacc(None, target_bir_lowering=False)

    x16_in = nc.dram_tensor("x16", [128, 4 * KC * 512], F16, kind="ExternalInput")
    w16_in = nc.dram_tensor("w16", [128, KC * CG], F16, kind="ExternalInput")
    qkb_in = nc.dram_tensor("qkb", [128, 4], F32, kind="ExternalInput")
    vb_in = nc.dram_tensor("vb", [1, 256], F16, kind="ExternalInput")
    wpa_in = nc.dram_tensor("wpa", [65, HPC * C], F16, kind="ExternalInput")
    out_part = nc.dram_tensor("out_part", [T // 4, C], F16, kind="ExternalOutput")

    partial_d = nc.dram_tensor("partial_d", [T, C], F16)
    rsout_d = [
        nc.dram_tensor(f"rsout_d{i}", [(r1 - r0) // 4, C], F16)
        for i, (r0, r1) in enumerate(RS_GROUPS)
    ]

    dbg = {}
    if DEBUG:
        dbg["qkT"] = nc.dram_tensor("dbg_qkT", [128, 4 * T], F16, kind="ExternalOutput")
        dbg["v_aug"] = nc.dram_tensor(
            "dbg_v_aug", [128, TT * HPC * 65], F16, kind="ExternalOutput"
        )
        dbg["oT"] = nc.dram_tensor("dbg_oT", [65, HPC * 512 * 2], F16, kind="ExternalOutput")
        dbg["partial"] = nc.dram_tensor("dbg_partial", [T, C], F16, kind="ExternalOutput")

    with tile.TileContext(nc) as tc:
        with (
            tc.tile_pool(name="cpool", bufs=1) as cpool,
            tc.tile_pool(name="main", bufs=1) as main,
            tc.tile_pool(name="stage", bufs=1) as stage,
            tc.tile_pool(name="ps", bufs=1, space="PSUM") as ps,
        ):
            # ---------------- constants ----------------
            ones_row = cpool.tile([1, 512], F16)
            nc.vector.memset(ones_row[:], 1.0)
            vb_bc = cpool.tile([128, 256], F16)
            # mask stationary: mstat[f, p] = -200 where p > f else 0
            mstat = cpool.tile([128, 128], F16)
            nc.gpsimd.memset(mstat[:], -200.0)
            nc.gpsimd.affine_select(
                out=mstat[:], in_=mstat[:],
                compare_op=mybir.AluOpType.is_ge, fill=0.0,
                base=-1, pattern=[[1, 128]], channel_multiplier=-1,
            )
            # mask moving: identity
            mmov = cpool.tile([128, 128], F16)
            nc.gpsimd.memset(mmov[:], 0.0)
            nc.gpsimd.affine_select(
                out=mmov[:], in_=mmov[:],
                compare_op=mybir.AluOpType.not_equal, fill=1.0,
                base=0, pattern=[[-1, 128]], channel_multiplier=1,
            )

            # ---------------- persistent tensors ----------------
            x16 = main.tile([128, 4 * KC * 512], F16)   # [w][kc][512]
            w16 = main.tile([128, KC * CG], F16)        # [kc][768]
            qkb = main.tile([128, 4], F32)
            vb = main.tile([1, 256], F16)
            wpa = main.tile([65, HPC * C], F16)
            qkT = main.tile([128, 4 * T], F16)             # [Q01;Q23;K01;K23] x T
            v_aug = main.tile([128, TT * HPC * 65], F16)   # per (tt,h): 64 V + ones col
            oT_sb = [
                main.tile([65, HPC * 512], F16, name=f"oT_sb{i}") for i in range(2)
            ]

            nc.vector.memset(v_aug[:], 1.0)  # ones columns give softmax rowsums
            for buf in oT_sb:
                nc.vector.memset(buf[64:65, :], 1.0)

            # ---------------- input DMAs (all cast-free, HWDGE) ----------
            w16_r = w16[:].rearrange("p (kc m) -> p kc m", kc=KC)
            w16_in_r = w16_in[:].rearrange("p (kc m) -> p kc m", kc=KC)
            nc.sync.dma_start(w16_r[:, 0:4, 512:768], w16_in_r[:, 0:4, 512:768])
            nc.sync.dma_start(x16[:, 0:2048], x16_in[:, 0:2048])
            nc.sync.dma_start(w16_r[:, 4:8, 512:768], w16_in_r[:, 4:8, 512:768])
            nc.sync.dma_start(x16[:, 2048:4096], x16_in[:, 2048:4096])
            nc.sync.dma_start(vb[:], vb_in[:])
            nc.gpsimd.partition_broadcast(vb_bc[:], vb[:])
            nc.sync.dma_start(qkb[:], qkb_in[:])
            nc.sync.dma_start(w16_r[:, :, 0:512], w16_in_r[:, :, 0:512])
            for w in range(1, 4):
                nc.sync.dma_start(
                    x16[:, w * 4096 : (w + 1) * 4096],
                    x16_in[:, w * 4096 : (w + 1) * 4096],
                )
            nc.sync.dma_start(wpa[:], wpa_in[:])

            # ---------------- emit helpers ----------------
            def x16_w(w):
                # [128, kc, 512] view of window w
                return x16[:, w * 4096 : (w + 1) * 4096].rearrange(
                    "p (kc t) -> p kc t", kc=KC
                )


            def emit_v(tt):
                w, tloc = divmod(tt, 4)
                pp = ps.tile([128, 1024], F32, tag="mm", bufs=3)
                for kc in range(KC):
                    nc.tensor.matmul(
                        pp[:, 0:256],
                        x16_w(w)[:, kc, tloc * 128 : (tloc + 1) * 128],
                        w16_r[:, kc, 512:768],
                        start=(kc == 0), stop=(kc == KC - 1),
                    )
                vt = v_aug[:, tt * HPC * 65 : (tt + 1) * HPC * 65].rearrange(
                    "p (h c) -> p h c", c=65
                )[:, :, 0:64]
                nc.vector.scalar_tensor_tensor(
                    out=vt,
                    in0=pp[:, 0:256].rearrange("p (h c) -> p h c", c=64),
                    scalar=1.0,
                    in1=vb_bc[:].rearrange("p (h c) -> p h c", c=64),
                    op0=mybir.AluOpType.mult,
                    op1=mybir.AluOpType.add,
                )

            def emit_qk(i, tch, evac_on_act=False):
                pp0 = ps.tile([128, 1024], F32, tag="mm", bufs=3)
                pp = pp0[:, 0:512]
                for kc in range(KC):
                    nc.tensor.matmul(
                        pp,
                        w16_r[:, kc, i * 128 : (i + 1) * 128],
                        x16_w(tch)[:, kc, :],
                        start=(kc == 0), stop=(kc == KC - 1),
                    )
                dst = qkT[:, i * T + tch * 512 : i * T + (tch + 1) * 512]
                if evac_on_act:
                    nc.scalar.activation(
                        dst, pp, mybir.ActivationFunctionType.Identity,
                        bias=qkb[:, i : i + 1],
                    )
                else:
                    nc.vector.tensor_scalar_add(dst, pp, qkb[:, i : i + 1])

            # per (sc, h) attention state
            ot_tiles = {}
            rs_tiles = {}

            def emit_att_head(sc, h, fillers=None):
                qT = qkT[64 * (h % 2) : 64 * (h % 2) + 64, (h // 2) * T : (h // 2 + 1) * T]
                kT = qkT[64 * (h % 2) : 64 * (h % 2) + 64, (2 + h // 2) * T : (3 + h // 2) * T]
                oT_ps = ps.tile([65, 512], F32, tag="ot", bufs=2)
                ot_tiles[(sc, h)] = oT_ps
                n_kj = (sc + 1) * 4
                npairs = n_kj // 2

                def pair_layout(p):
                    # [(bank_off, q_off, cols, kj), ...]; pack both k tiles
                    # into one PSUM bank when their columns fit (saves exp
                    # span and a bank)
                    kj0, kj1 = 2 * p, 2 * p + 1
                    qo0 = max(0, kj0 * 128 - sc * 512)
                    qo1 = max(0, kj1 * 128 - sc * 512)
                    c0, c1 = 512 - qo0, 512 - qo1
                    if c0 + c1 <= 512:
                        return [(0, qo0, c0, kj0), (c0, qo1, c1, kj1)], c0 + c1
                    return [(0, qo0, c0, kj0), (512, qo1, c1, kj1)], 512 + c1

                def emit_s_pair(p):
                    layout, span = pair_layout(p)
                    one_bank = layout[1][0] < 512
                    st = ps.tile([128, 1024], F32, tag="mm", bufs=3)
                    pt = stage.tile([128, 1024], F16, tag="pt", bufs=4)
                    for idx, (boff, q_off, cols, kj) in enumerate(layout):
                        diag = kj >= sc * 4
                        first = idx == 0 or not one_bank
                        last_in_group = (not one_bank) or idx == 1
                        nc.tensor.matmul(
                            st[:, boff : boff + cols],
                            kT[:, kj * 128 : (kj + 1) * 128],
                            qT[:, sc * 512 + q_off : (sc + 1) * 512],
                            start=first,
                            stop=(not diag) and last_in_group,
                            skip_group_check=not first,
                        )
                        if diag:
                            nc.tensor.matmul(
                                st[:, boff : boff + 128], mstat[:], mmov[:],
                                start=False, stop=last_in_group,
                                skip_group_check=True,
                            )
                    # one exp covering both halves (cols between valid ranges
                    # hold stale PSUM; the pt garbage there is never read)
                    nc.scalar.activation(
                        pt[:, :span], st[:, :span],
                        mybir.ActivationFunctionType.Exp,
                        scale=EXP_SCALE,
                    )
                    return pt, layout

                def emit_pv_pair(p, pt, layout):
                    for boff, q_off, cols, kj in layout:
                        vv = v_aug[:, (kj * HPC + h) * 65 : (kj * HPC + h + 1) * 65]
                        nc.tensor.matmul(
                            oT_ps[:, q_off:512],
                            vv,
                            pt[:, boff : boff + cols],
                            start=(kj == 0),
                            stop=(kj == n_kj - 1),
                        )

                # software pipeline: emit S(p+1) before PV(p) so PE always has
                # matmul work queued while exp(p) completes on ACT; fillers
                # (independent matmul units) slot in to absorb ACT-bound gaps
                fillers = list(fillers or [])
                pend = emit_s_pair(0)
                for p in range(1, npairs):
                    nxt = emit_s_pair(p)
                    if fillers:
                        fillers.pop(0)()
                    emit_pv_pair(p - 1, *pend)
                    pend = nxt
                emit_pv_pair(npairs - 1, *pend)
                for f in fillers:
                    f()
                # normalize chain for this head (DVE + Pool), frees oT_ps
                rs = stage.tile([1, 512], F32, tag="rs", bufs=4)
                nc.vector.tensor_copy(rs[:], oT_ps[64:65, :])
                recip = stage.tile([1, 512], F32, tag="recip", bufs=4)
                nc.vector.reciprocal_approx_fast(recip[:], rs[:])
                bc = stage.tile([64, 512], F32, tag="bc", bufs=4)
                nc.gpsimd.partition_broadcast(bc[:], recip[:])
                nc.vector.tensor_mul(
                    oT_sb[sc % 2][0:64, h * 512 : (h + 1) * 512],
                    oT_ps[0:64, :], bc[:],
                )

            def emit_proj_piece(sc, j):
                # project rows [sc*512 + j*128, +128)
                r0 = sc * 512 + j * 128
                oT_cur = oT_sb[sc % 2]
                pp = ps.tile([128, 1024], F32, tag="mm", bufs=3)
                for nch in range(2):
                    for hh in range(HPC):
                        nc.tensor.matmul(
                            pp[:, nch * 512 : (nch + 1) * 512],
                            oT_cur[:, hh * 512 + j * 128 : hh * 512 + (j + 1) * 128],
                            wpa[:, hh * C + nch * 512 : hh * C + (nch + 1) * 512],
                            start=(hh == 0), stop=(hh == HPC - 1),
                        )
                pst = stage.tile([128, 1024], F16, tag="pst", bufs=3)
                nc.vector.tensor_copy(pst[:], pp[:])
                # spread partial-write issues across SEQ queues: each DMA
                # issue occupies its queue ~650ns and the tail needs several
                # in flight at once
                dq = [nc.sync, nc.scalar][j % 2]
                dq.dma_start(partial_d[r0 : r0 + 128, :], pst[:])

            def emit_rs(gi):
                r0, r1 = RS_GROUPS[gi]
                nc.gpsimd.collective_compute(
                    "ReduceScatter",
                    mybir.AluOpType.add,
                    replica_groups=[[0, 1, 2, 3], [4, 5, 6, 7]],
                    ins=[partial_d[r0:r1, :]],
                    outs=[rsout_d[gi][:]],
                )

            def emit_out_copy(gi):
                # deferred to the tail: an out-copy waiting on its collective
                # must not sit in an in-order DMA queue ahead of partial
                # writes that later collectives depend on
                r0, r1 = RS_GROUPS[gi]
                og = sum((b1 - b0) // 4 for (b0, b1) in RS_GROUPS[:gi])
                ln4 = (r1 - r0) // 4
                nc.sync.dma_start(out_part[og : og + ln4, :], rsout_d[gi][:])

            # ---------------- schedule ----------------
            for tt in range(8):
                emit_v(tt)
            for tch in (0, 1):
                for i in range(4):
                    emit_qk(i, tch, evac_on_act=True)

            # att sc0 (V8/V9 fill the tiny heads' chain latency)
            att0_fill = [None, None, [lambda: emit_v(8)], [lambda: emit_v(9)]]
            for h in range(HPC):
                emit_att_head(0, h, att0_fill[h])
            # att sc1 with proj(sc0) interleaved between heads
            att1_fill = [[lambda: emit_v(10)], [lambda: emit_v(11)], None, None]
            for h in range(HPC):
                emit_att_head(1, h, att1_fill[h])
                emit_proj_piece(0, h)

            # fin1 (stall filled by two tch2 QK units), then RS group 0
            emit_qk(0, 2)
            emit_qk(1, 2)
            for j in range(4):
                emit_proj_piece(1, j)
            emit_rs(0)
            emit_qk(2, 2)
            emit_qk(3, 2)

            # att2 with att3's prerequisites sprinkled into exp-wait slots
            att2_fill = [
                [lambda: emit_v(12), lambda: emit_qk(0, 3)],
                [lambda: emit_v(13), lambda: emit_qk(1, 3)],
                [lambda: emit_v(14)],
                [lambda: emit_v(15)],
            ]
            for h in range(HPC):
                emit_att_head(2, h, att2_fill[h])
            # fin2 start (two tch3 QK units fill the chain), rest in att3
            emit_qk(2, 3)
            emit_qk(3, 3)
            emit_proj_piece(2, 0)
            emit_proj_piece(2, 1)

            for h in range(HPC):
                emit_att_head(3, h)
                if h == 0:
                    emit_proj_piece(2, 2)
                elif h == 1:
                    emit_proj_piece(2, 3)
                    emit_rs(1)
            for j in range(4):
                emit_proj_piece(3, j)
            emit_rs(2)
            for gi in range(len(RS_GROUPS)):
                emit_out_copy(gi)

            if DEBUG:
                nc.sync.dma_start(dbg["qkT"][:], qkT[:])
                nc.sync.dma_start(dbg["v_aug"][:], v_aug[:])
                nc.sync.dma_start(dbg["oT"][:, 0 : HPC * 512], oT_sb[0][:])
                nc.sync.dma_start(dbg["oT"][:, HPC * 512 :], oT_sb[1][:])
                nc.sync.dma_start(dbg["partial"][:], partial_d[:])

    nc.finalize()
    return nc


_NC = None


def _get_nc():
    global _NC
    if _NC is None:
        _NC = _build()
    return _NC


def _perm_qkv(w):
    # (..., h*192 + t*64 + c) -> (..., t*256 + h*64 + c)
    s = w.shape[:-1]
    return np.ascontiguousarray(
        w.reshape(*s, HPC, 3, HD).swapaxes(-3, -2).reshape(*s, CG)
    )


def _make_in_maps(x, Wqkv, bqkv, Wproj, bproj):
    x = np.asarray(x, dtype=np.float32)
    Wqkv = np.asarray(Wqkv, dtype=np.float32)
    bqkv = np.asarray(bqkv, dtype=np.float32)
    Wproj = np.asarray(Wproj, dtype=np.float32)
    bproj = np.asarray(bproj, dtype=np.float32)

    in_maps = []
    for c in range(N_CORES):
        b, g = divmod(c, 4)
        # x16: [p, w, kc, t] window-major transposed layout
        xT = x[b].T  # (C, T)
        x16 = np.ascontiguousarray(
            xT.reshape(KC, 128, 4, 512).transpose(1, 2, 0, 3).reshape(128, -1)
        ).astype(np.float16)
        wp_ = _perm_qkv(Wqkv[:, g * CG : (g + 1) * CG])
        w16 = np.ascontiguousarray(
            wp_.reshape(KC, 128, CG).transpose(1, 0, 2).reshape(128, -1)
        ).astype(np.float16)
        bq = _perm_qkv(bqkv[g * CG : (g + 1) * CG])
        qkb = np.ascontiguousarray(bq[:512].reshape(4, 128).T).astype(np.float32)
        vb = bq[512:768].reshape(1, 256).astype(np.float16)
        wpa = np.zeros((65, HPC * C), np.float32)
        for hh in range(HPC):
            wpa[0:64, hh * C : (hh + 1) * C] = Wproj[
                g * 256 + hh * 64 : g * 256 + (hh + 1) * 64, :
            ]
        if g == 0:
            wpa[64, 0:C] = bproj
        in_maps.append(
            {
                "x16": x16,
                "w16": w16,
                "qkb": qkb,
                "vb": vb,
                "wpa": wpa.astype(np.float16),
            }
        )
    return in_maps


def _run(in_maps, trace=False):
    nc = _get_nc()
    return run_bass_kernel_spmd(nc, in_maps, list(range(N_CORES)), trace=trace)


def kernel(x, Wqkv, bqkv, Wproj, bproj):
    in_maps = _make_in_maps(x, Wqkv, bqkv, Wproj, bproj)
    res = _run(in_maps)
    out = np.empty((B, T, C), np.float32)
    for c in range(N_CORES):
        b, g = divmod(c, 4)
        op = res.results[c]["out_part"].astype(np.float32)
        og = 0
        for r0, r1 in RS_GROUPS:
            ln4 = (r1 - r0) // 4
            out[b, r0 + g * ln4 : r0 + (g + 1) * ln4, :] = op[og : og + ln4]
            og += ln4
    return out
